# revision 1
# baseline (speedup 1.0000x reference)
"""GCN link predictor on 8 Trainium2 NeuronCores (Bass/Tile).

Strategy (all compute on device; host does index preprocessing only):
  - Nodes sharded 12500/core (padded to 12800 = 25 supertiles of 512).
  - Phase A: per-shard G1 = x @ W1 + b1 (node-major), AllGather -> G1 full.
  - Phase B: L1 aggregation. Edges partitioned by destination core; per core,
    grouped by (supertile, col-segment, 64-dest window) into 128-edge chunks
    (uniform chunk structure across cores via max-over-cores quotas so one
    SPMD program fits all 8 cores).  Per chunk: dma_gather 512B rows of G1
    (int16 indices into one of 4 row-segments), fused DVE tensor_scalar
    builds eq_val[e, j] = val_e * (dest_rel_e == j), PE matmul accumulates
    agg_T[feat, dest] windows in PSUM.  Epilogue per supertile: relu (ACT),
    g2_T = W2^T @ r1_T + b2, PE-transpose to node-major, DMA out.
    AllGather -> G2 full.
  - Phase C: L2 aggregation, same chunk structure, gather 256B rows of G2,
    z_T windows in PSUM, transpose, DMA, AllGather -> Z full.
  - Phase D: decode. Queries sharded 25000/core, grouped by
    (src-segment, dst-segment) with quotas; dma_gather z[src], z[dst],
    DVE multiply + reduce -> dot products.
"""
import sys
sys.path.insert(0, '/opt/trn_rl_repo')
import numpy as np
import concourse.bass as bass
import concourse.bacc as bacc
import concourse.mybir as mybir
import concourse.tile as tile
from concourse.bass_utils import run_bass_kernel_spmd
from concourse.masks import make_identity

# problem constants (hardcoded per task contract)
N = 100000
IN_F, HID, OUT = 256, 128, 64
NNZ = 1600000
EQ = 200000

NCORES = 8
SHR = 12500                 # real nodes per core
SH = 12800                  # padded nodes per core
NPAD = SH * NCORES          # 102400
SEG_R = 25600               # rows per gather segment (int16-safe)
NSEG = 4
WIN = 64                    # dest window width
WPS = 8                     # windows per supertile
ST = WIN * WPS              # 512 dests per supertile
NST = SH // ST              # 25 supertiles
NW = SH // WIN              # 200 windows per core
P = 128
EQC = EQ // NCORES          # 25000 queries per core

FP = mybir.dt.float32
I16 = mybir.dt.int16
GMAX = 8            # max chunks (x128 idx) per dma_gather instruction
DSCRATCH = 131072   # SWDGE descriptor scratch carveout


def _wrap_idx(flat):
    """int16 flat index list (len % 128 == 0) -> [128, len//16] wrapped/replicated."""
    n = flat.shape[0]
    w = flat.reshape(n // 16, 16).T  # [16, n//16]
    return np.tile(w, (8, 1)).copy()


def _preprocess(inputs):
    x = np.ascontiguousarray(np.asarray(inputs["x"], dtype=np.float32))
    adj_rows = np.asarray(inputs["adj_rows"], dtype=np.int64)
    adj_cols = np.asarray(inputs["adj_cols"], dtype=np.int64)
    adj_vals = np.asarray(inputs["adj_vals"], dtype=np.float32)
    edge_index = np.asarray(inputs["edge_index"], dtype=np.int64)
    W1 = np.asarray(inputs["W1"], dtype=np.float32)
    b1 = np.asarray(inputs["b1"], dtype=np.float32)
    W2 = np.asarray(inputs["W2"], dtype=np.float32)
    b2 = np.asarray(inputs["b2"], dtype=np.float32)

    # ---- per-core edge partition ----
    bounds = np.searchsorted(adj_rows, np.arange(NCORES + 1) * SHR)
    cores = []
    counts = np.zeros((NCORES, NW * NSEG), dtype=np.int64)
    for r in range(NCORES):
        b0, b1e = bounds[r], bounds[r + 1]
        d_loc = adj_rows[b0:b1e] - r * SHR
        cols = adj_cols[b0:b1e]
        vals = adj_vals[b0:b1e]
        pid = (cols // SHR) * SH + (cols % SHR)
        seg = pid // SEG_R
        sidx = (pid % SEG_R).astype(np.int64)
        win = d_loc // WIN
        key = win * NSEG + seg
        order = np.lexsort((sidx, key))
        cores.append((d_loc[order], sidx[order], vals[order], key[order]))
        counts[r] = np.bincount(key, minlength=NW * NSEG)

    # quotas: chunks per (window, segment) cell, uniform across cores
    quota = -(-counts.max(axis=0) // P)          # ceil, [NW*NSEG]
    qw = quota.reshape(NW, NSEG)
    for w in range(NW):
        if qw[w].sum() == 0:
            qw[w, 0] = 1   # >=1 chunk per window so its psum columns get written

    # chunk order: supertile-major, then segment, then window
    # chunk list entries: (window, segment)
    chunk_win = []
    chunk_seg = []
    st_seg_nchunks = np.zeros((NST, NSEG), dtype=np.int64)
    st_first = {}
    st_last = {}
    for st in range(NST):
        for s in range(NSEG):
            for w in range(st * WPS, (st + 1) * WPS):
                for _ in range(qw[w, s]):
                    ci = len(chunk_win)
                    chunk_win.append(w)
                    chunk_seg.append(s)
                    if st not in st_first:
                        st_first[st] = ci
                    st_last[st] = ci
                    st_seg_nchunks[st, s] += 1
    NCH = len(chunk_win)
    chunk_win = np.array(chunk_win)
    chunk_seg = np.array(chunk_seg)
    chunk_start = np.zeros(NCH, dtype=bool)
    chunk_stop = np.zeros(NCH, dtype=bool)
    for st, ci in st_first.items():
        chunk_start[ci] = True
    for st, ci in st_last.items():
        chunk_stop[ci] = True

    # chunk base index per (window, segment) cell in the global chunk order
    cell_chunk_base = np.zeros((NW, NSEG), dtype=np.int64)
    pos = 0
    for st in range(NST):
        for s in range(NSEG):
            for w in range(st * WPS, (st + 1) * WPS):
                cell_chunk_base[w, s] = pos
                pos += qw[w, s]
    assert pos == NCH

    # ---- per-core slot arrays ----
    cols16_all = np.zeros((NCORES, NCH, P), dtype=np.int16)
    dest_all = np.zeros((NCORES, NCH, P), dtype=np.float32)
    val_all = np.zeros((NCORES, NCH, P), dtype=np.float32)
    for r in range(NCORES):
        d_loc, sidx, vals, key = cores[r]
        ne = d_loc.shape[0]
        if ne == 0:
            continue
        cnt = counts[r]
        starts = np.zeros(NW * NSEG + 1, dtype=np.int64)
        np.cumsum(cnt, out=starts[1:])
        rank = np.arange(ne) - starts[key]
        w_arr = key // NSEG
        s_arr = key % NSEG
        chunk_id = cell_chunk_base[w_arr, s_arr] + rank // P
        slot = rank % P
        cols16_all[r, chunk_id, slot] = sidx.astype(np.int16)
        dest_all[r, chunk_id, slot] = (d_loc - w_arr * WIN).astype(np.float32)
        val_all[r, chunk_id, slot] = vals

    # gather groups: one dma_gather per (supertile, segment) with
    # num_idxs = st_seg_nchunks[st, s] * 128 (skip if 0)
    # idx arrays per core, wrapped: concatenated in chunk order
    idx_wrapped = np.zeros((NCORES, P, NCH * P // 16), dtype=np.int16)
    for r in range(NCORES):
        flat = cols16_all[r].reshape(NCH * P)
        idx_wrapped[r] = _wrap_idx(flat)

    # dest/val arrays laid out [P, NCH] for per-chunk [128,1] scalar slices
    destv = dest_all.transpose(0, 2, 1).copy()   # [NCORES, 128, NCH]
    vh = val_all.astype(np.float16).astype(np.float32)
    vl = (val_all - vh).astype(np.float16).astype(np.float32)
    valhi = vh.transpose(0, 2, 1).copy()
    vallo = vl.transpose(0, 2, 1).copy()

    # ---- decode preprocessing ----
    src = edge_index[0]
    dst = edge_index[1]
    spid = (src // SHR) * SH + (src % SHR)
    dpid = (dst // SHR) * SH + (dst % SHR)
    dec = []
    dcounts = np.zeros((NCORES, NSEG * NSEG), dtype=np.int64)
    for r in range(NCORES):
        q0, q1 = r * EQC, (r + 1) * EQC
        ss = spid[q0:q1] // SEG_R
        ds = dpid[q0:q1] // SEG_R
        gkey = ss * NSEG + ds
        order = np.lexsort((np.arange(EQC), gkey))
        dec.append((spid[q0:q1] % SEG_R, dpid[q0:q1] % SEG_R, gkey, order))
        dcounts[r] = np.bincount(gkey, minlength=NSEG * NSEG)
    dquota = -(-dcounts.max(axis=0) // P)         # chunks per (ss,ds) group
    NQCH = int(dquota.sum())
    dbase = np.concatenate([[0], np.cumsum(dquota)])[:-1]  # chunk base per group

    sidx_dec = np.zeros((NCORES, NQCH, P), dtype=np.int16)
    didx_dec = np.zeros((NCORES, NQCH, P), dtype=np.int16)
    perm_dec = np.full((NCORES, NQCH, P), -1, dtype=np.int64)  # slot -> query id
    for r in range(NCORES):
        sloc, dloc, gkey, order = dec[r]
        gk = gkey[order]
        cnt = dcounts[r]
        starts = np.zeros(NSEG * NSEG + 1, dtype=np.int64)
        np.cumsum(cnt, out=starts[1:])
        rank = np.arange(EQC) - starts[gk]
        chunk_id = dbase[gk] + rank // P
        slot = rank % P
        sidx_dec[r, chunk_id, slot] = sloc[order].astype(np.int16)
        didx_dec[r, chunk_id, slot] = dloc[order].astype(np.int16)
        perm_dec[r, chunk_id, slot] = order
    sidx_wr = np.zeros((NCORES, P, NQCH * P // 16), dtype=np.int16)
    didx_wr = np.zeros((NCORES, P, NQCH * P // 16), dtype=np.int16)
    for r in range(NCORES):
        sidx_wr[r] = _wrap_idx(sidx_dec[r].reshape(NQCH * P))
        didx_wr[r] = _wrap_idx(didx_dec[r].reshape(NQCH * P))

    # decode gather groups: src gathers per ss-run, dst gathers per (ss,ds)
    dq = dquota.reshape(NSEG, NSEG)
    src_runs = []   # (chunk_base, nchunks, seg)
    dst_runs = []
    pos = 0
    for ss in range(NSEG):
        n = int(dq[ss].sum())
        if n:
            src_runs.append((pos, n, ss))
        p2 = pos
        for ds in range(NSEG):
            if dq[ss, ds]:
                dst_runs.append((p2, int(dq[ss, ds]), ds))
            p2 += int(dq[ss, ds])
        pos += n

    # ---- dense inputs ----
    xt_full = x.T  # [256, 100000]
    in_maps = []
    iota64 = np.broadcast_to(np.arange(WIN, dtype=np.float32), (P, WIN)).copy()
    b1b = np.broadcast_to(b1, (P, HID)).copy()
    for r in range(NCORES):
        xT_sh = np.zeros((IN_F, SH), dtype=np.float32)
        xT_sh[:, :SHR] = xt_full[:, r * SHR:(r + 1) * SHR]
        in_maps.append({
            "xT": xT_sh,
            "W1": W1.copy(),
            "b1b": b1b.copy(),
            "W2": W2.copy(),
            "b2c": b2.reshape(OUT, 1).copy(),
            "iota64": iota64.copy(),
            "colsw": idx_wrapped[r],
            "destv": destv[r],
            "valhi": valhi[r],
            "vallo": vallo[r],
            "qsrcw": sidx_wr[r],
            "qdstw": didx_wr[r],
        })

    meta = {
        "NCH": NCH,
        "chunk_win": chunk_win,
        "chunk_seg": chunk_seg,
        "chunk_start": chunk_start,
        "chunk_stop": chunk_stop,
        "st_seg_nchunks": st_seg_nchunks,
        "NQCH": NQCH,
        "src_runs": src_runs,
        "dst_runs": dst_runs,
        "perm_dec": perm_dec,
    }
    return in_maps, meta


def _build(meta, single=False, upto='full'):
    NCH = meta["NCH"]
    chunk_win = meta["chunk_win"]
    chunk_seg = meta["chunk_seg"]
    chunk_start = meta["chunk_start"]
    chunk_stop = meta["chunk_stop"]
    ssn = meta["st_seg_nchunks"]
    NQCH = meta["NQCH"]
    src_runs = meta["src_runs"]
    dst_runs = meta["dst_runs"]

    ncore = 1 if single else NCORES
    nc = bacc.Bacc("TRN2", target_bir_lowering=False, debug=False,
                   num_devices=ncore, dynamic_dma_scratch_size=DSCRATCH,
                   num_swdge_queues=4)
    qrr = [0]

    def _next_q():
        qrr[0] = (qrr[0] + 1) % 4
        return qrr[0]

    def _collective(name, in_ap, out_handle):
        if single:
            nc.sync.dma_start(
                out=out_handle[0:SH, :].opt(),
                in_=in_ap.opt())
        else:
            nc.gpsimd.collective_compute(
                "AllGather", mybir.AluOpType.bypass, replica_groups=rg,
                ins=[in_ap], outs=[out_handle[:]])

    t_xT = nc.dram_tensor("xT", [IN_F, SH], FP, kind="ExternalInput")
    t_W1 = nc.dram_tensor("W1", [IN_F, HID], FP, kind="ExternalInput")
    t_b1b = nc.dram_tensor("b1b", [P, HID], FP, kind="ExternalInput")
    t_W2 = nc.dram_tensor("W2", [HID, OUT], FP, kind="ExternalInput")
    t_b2c = nc.dram_tensor("b2c", [OUT, 1], FP, kind="ExternalInput")
    t_iota = nc.dram_tensor("iota64", [P, WIN], FP, kind="ExternalInput")
    t_cols = nc.dram_tensor("colsw", [P, NCH * P // 16], I16, kind="ExternalInput")
    t_destv = nc.dram_tensor("destv", [P, NCH], FP, kind="ExternalInput")
    t_valhi = nc.dram_tensor("valhi", [P, NCH], FP, kind="ExternalInput")
    t_vallo = nc.dram_tensor("vallo", [P, NCH], FP, kind="ExternalInput")
    t_qsrc = nc.dram_tensor("qsrcw", [P, NQCH * P // 16], I16, kind="ExternalInput")
    t_qdst = nc.dram_tensor("qdstw", [P, NQCH * P // 16], I16, kind="ExternalInput")

    o_dec = nc.dram_tensor("out", [P, NQCH], FP, kind="ExternalOutput")

    F16 = mybir.dt.float16
    g1_local = nc.dram_tensor("g1_local", [SH, 2 * HID], F16)
    G1 = nc.dram_tensor("G1full", [NPAD, 2 * HID], F16, addr_space="Shared")
    g2_local = nc.dram_tensor("g2_local", [NST, 4, P, 4 * OUT], F16)
    G2 = nc.dram_tensor("G2full", [NPAD, 4 * OUT], F16, addr_space="Shared")
    z_local = nc.dram_tensor("z_local", [NST, 4, P, OUT], FP)
    Z = nc.dram_tensor("Zfull", [NPAD, OUT], FP, addr_space="Shared")

    rg = [list(range(NCORES))]

    with tile.TileContext(nc) as tc:
        with tc.tile_pool(name="const", bufs=1) as cp:
            w1a = cp.tile([P, HID], FP)
            nc.sync.dma_start(out=w1a[:], in_=t_W1[0:P, :])
            w1b = cp.tile([P, HID], FP)
            nc.sync.dma_start(out=w1b[:], in_=t_W1[P:IN_F, :])
            b1t = cp.tile([P, HID], FP)
            nc.sync.dma_start(out=b1t[:], in_=t_b1b[:])
            w2t = cp.tile([HID, OUT], FP)
            nc.sync.dma_start(out=w2t[:], in_=t_W2[:])
            b2t = cp.tile([OUT, 1], FP)
            nc.sync.dma_start(out=b2t[:], in_=t_b2c[:])
            iota_t = cp.tile([P, WIN], FP)
            nc.sync.dma_start(out=iota_t[:], in_=t_iota[:])
            id64 = cp.tile([WIN, WIN], FP)
            make_identity(nc, id64[:])

            # ================= Phase A: G1 = x @ W1 + b1 =================
            with nc.named_scope("phaseA"):
                with (tc.tile_pool(name="xa", bufs=3) as xa,
                      tc.tile_pool(name="g1s", bufs=3) as g1s,
                      tc.tile_pool(name="psA", bufs=2, space="PSUM") as psA):
                    for t in range(SH // P):
                        xlo = xa.tile([P, P], FP, tag="xlo")
                        nc.sync.dma_start(out=xlo[:], in_=t_xT[0:P, t * P:(t + 1) * P])
                        xhi = xa.tile([P, P], FP, tag="xhi")
                        nc.sync.dma_start(out=xhi[:], in_=t_xT[P:IN_F, t * P:(t + 1) * P])
                        pt = psA.tile([P, HID], FP, space="PSUM")
                        nc.tensor.matmul(out=pt[:], lhsT=xlo[:], rhs=w1a[:],
                                         start=True, stop=False)
                        nc.tensor.matmul(out=pt[:], lhsT=xhi[:], rhs=w1b[:],
                                         start=False, stop=True)
                        gt = g1s.tile([P, HID], FP, tag="g1")
                        nc.vector.tensor_tensor(out=gt[:], in0=pt[:], in1=b1t[:],
                                                op=mybir.AluOpType.add)
                        hl = g1s.tile([P, 2 * HID], F16, tag="g1hl")
                        nc.vector.tensor_copy(out=hl[:, 0:HID], in_=gt[:])
                        nc.vector.tensor_tensor(out=hl[:, HID:2 * HID], in0=gt[:],
                                                in1=hl[:, 0:HID],
                                                op=mybir.AluOpType.subtract)
                        nc.sync.dma_start(out=g1_local[t * P:(t + 1) * P, :], in_=hl[:])

                _collective("ag1", g1_local[:], G1)

            # ============ Phase B / C: aggregation layers ============
            def agg_layer(layer, table, feat, out_local, do_g2):
                # table rows: [hi(feat) | lo(feat) | pad] fp16
                row_elems = 2 * feat if feat == HID else 4 * feat
                scope = f"agg{layer}"
                with nc.named_scope(scope):
                    with (tc.tile_pool(name=f"gi{layer}", bufs=4) as gi,
                          tc.tile_pool(name=f"gm{layer}", bufs=5) as gm,
                          tc.tile_pool(name=f"dv{layer}", bufs=4) as dvp,
                          tc.tile_pool(name=f"eq{layer}", bufs=24) as eqp,
                          tc.tile_pool(name=f"ep{layer}", bufs=2) as ep,
                          tc.tile_pool(name=f"ps{layer}", bufs=2, space="PSUM") as psp,
                          tc.tile_pool(name=f"pg{layer}", bufs=2, space="PSUM") as pgp,
                          tc.tile_pool(name=f"pt{layer}", bufs=2, space="PSUM") as ptp):
                        ci = 0
                        for st in range(NST):
                            pst = psp.tile([P, ST], FP, space="PSUM", tag="agg")
                            for s in range(NSEG):
                                nch_all = int(ssn[st, s])
                                if nch_all == 0:
                                    continue
                                ci0 = ci
                                it = gi.tile([P, nch_all * P // 16], I16, tag="idx")
                                nc.sync.dma_start(
                                    out=it[:],
                                    in_=t_cols[:, ci0 * P // 16:(ci0 + nch_all) * P // 16])
                                dvt = dvp.tile([P, nch_all], FP, tag="dest")
                                nc.sync.dma_start(
                                    out=dvt[:], in_=t_destv[:, ci0:ci0 + nch_all])
                                vht = dvp.tile([P, nch_all], FP, tag="valhi")
                                nc.sync.dma_start(
                                    out=vht[:], in_=t_valhi[:, ci0:ci0 + nch_all])
                                vlt = dvp.tile([P, nch_all], FP, tag="vallo")
                                nc.sync.dma_start(
                                    out=vlt[:], in_=t_vallo[:, ci0:ci0 + nch_all])
                                for g0 in range(0, nch_all, GMAX):
                                    nch = min(GMAX, nch_all - g0)
                                    nidx = nch * P
                                    gt = gm.tile([P, nch, row_elems], F16, tag="msgs")
                                    nc.gpsimd.dma_gather(
                                        out_ap=gt[:],
                                        in_ap=table[s * SEG_R:(s + 1) * SEG_R, :],
                                        idxs_ap=it[:, g0 * P // 16:(g0 + nch) * P // 16],
                                        num_idxs=nidx, num_idxs_reg=nidx,
                                        elem_size=row_elems, queue_num=_next_q())
                                    for k in range(nch):
                                        c = ci + k
                                        kk = ci + k - ci0
                                        w = int(chunk_win[c])
                                        wc = (w % WPS) * WIN
                                        st_flag = bool(chunk_start[c])
                                        sp_flag = bool(chunk_stop[c])
                                        eh = eqp.tile([P, WIN], F16, tag="eqh")
                                        nc.vector.tensor_scalar(
                                            out=eh[:], in0=iota_t[:],
                                            scalar1=dvt[:, kk:kk + 1],
                                            scalar2=vht[:, kk:kk + 1],
                                            op0=mybir.AluOpType.is_equal,
                                            op1=mybir.AluOpType.mult)
                                        el = eqp.tile([P, WIN], F16, tag="eql")
                                        nc.vector.tensor_scalar(
                                            out=el[:], in0=iota_t[:],
                                            scalar1=dvt[:, kk:kk + 1],
                                            scalar2=vlt[:, kk:kk + 1],
                                            op0=mybir.AluOpType.is_equal,
                                            op1=mybir.AluOpType.mult)
                                        hi = gt[:, k, 0:feat]
                                        lo = gt[:, k, feat:2 * feat]
                                        nc.tensor.matmul(
                                            out=pst[:feat, wc:wc + WIN],
                                            lhsT=hi, rhs=eh[:],
                                            start=st_flag, stop=False)
                                        nc.tensor.matmul(
                                            out=pst[:feat, wc:wc + WIN],
                                            lhsT=lo, rhs=eh[:],
                                            start=False, stop=False)
                                        nc.tensor.matmul(
                                            out=pst[:feat, wc:wc + WIN],
                                            lhsT=hi, rhs=el[:],
                                            start=False, stop=sp_flag)
                                    ci += nch
                            # epilogue for supertile st
                            if do_g2:
                                rt = ep.tile([P, ST], FP, tag="r1")
                                nc.scalar.activation(
                                    out=rt[:], in_=pst[:],
                                    func=mybir.ActivationFunctionType.Relu)
                                pg = pgp.tile([OUT, ST], FP, space="PSUM", tag="g2")
                                nc.tensor.matmul(out=pg[:], lhsT=w2t[:], rhs=rt[:],
                                                 start=True, stop=True)
                                g2sb = ep.tile([OUT, ST], FP, tag="g2sb")
                                nc.vector.tensor_scalar(
                                    out=g2sb[:], in0=pg[:], scalar1=b2t[:],
                                    scalar2=None, op0=mybir.AluOpType.add)
                                src_t = g2sb
                            else:
                                zsb = ep.tile([OUT, ST], FP, tag="zsb")
                                nc.vector.tensor_copy(out=zsb[:], in_=pst[:OUT, :])
                                src_t = zsb
                            if do_g2:
                                stage = ep.tile([P, 4, 4 * OUT], F16, tag="stage")
                                nc.vector.memset(stage[:, :, 2 * OUT:4 * OUT], 0)
                                stf = ep.tile([P, 4, OUT], FP, tag="stagef")
                                for j in range(4):
                                    tp = ptp.tile([P, OUT], FP, space="PSUM", tag="tp")
                                    nc.tensor.transpose(
                                        out=tp[:], in_=src_t[:, j * P:(j + 1) * P],
                                        identity=id64[:])
                                    nc.vector.tensor_copy(out=stf[:, j, :], in_=tp[:])
                                    nc.vector.tensor_copy(
                                        out=stage[:, j, 0:OUT], in_=stf[:, j, :])
                                    nc.vector.tensor_tensor(
                                        out=stage[:, j, OUT:2 * OUT],
                                        in0=stf[:, j, :], in1=stage[:, j, 0:OUT],
                                        op=mybir.AluOpType.subtract)
                                nc.sync.dma_start(
                                    out=out_local[st].transpose([1, 0, 2]), in_=stage[:])
                            else:
                                stage = ep.tile([P, 4, OUT], FP, tag="stagez")
                                for j in range(4):
                                    tp = ptp.tile([P, OUT], FP, space="PSUM", tag="tp")
                                    nc.tensor.transpose(
                                        out=tp[:], in_=src_t[:, j * P:(j + 1) * P],
                                        identity=id64[:])
                                    nc.vector.tensor_copy(out=stage[:, j, :], in_=tp[:])
                                nc.sync.dma_start(
                                    out=out_local[st].transpose([1, 0, 2]), in_=stage[:])

            if upto != 'A':
                agg_layer(1, G1, HID, g2_local, do_g2=True)
                with nc.named_scope("ag2"):
                    _collective("ag2", g2_local[:].opt(), G2)
            if upto in ('L2', 'full'):
                agg_layer(2, G2, OUT, z_local, do_g2=False)
                with nc.named_scope("ag3"):
                    _collective("ag3", z_local[:].opt(), Z)

            # ================= Phase D: decode =================
            if upto != 'full':
                pass
            else:
             with nc.named_scope("decode"):
                with (tc.tile_pool(name="qs", bufs=2) as qs,
                      tc.tile_pool(name="qd", bufs=3) as qd,
                      tc.tile_pool(name="qi", bufs=4) as qi,
                      tc.tile_pool(name="qo", bufs=3) as qo):
                    red_all = qo.tile([P, NQCH], FP, tag="redall")

                    def gath(pool, tag, tdram, seg, cb, nch, ipool, itag, idram):
                        t = pool.tile([P, nch, OUT], FP, tag=tag)
                        for g0 in range(0, nch, GMAX):
                            n = min(GMAX, nch - g0)
                            it = ipool.tile([P, n * P // 16], I16, tag=itag)
                            o16 = (cb + g0) * P // 16
                            nc.sync.dma_start(
                                out=it[:], in_=idram[:, o16:o16 + n * P // 16])
                            nc.gpsimd.dma_gather(
                                out_ap=t[:, g0:g0 + n, :],
                                in_ap=Z[seg * SEG_R:(seg + 1) * SEG_R, :],
                                idxs_ap=it[:], num_idxs=n * P, num_idxs_reg=n * P,
                                elem_size=OUT, queue_num=_next_q())
                        return t

                    for (cb, nch, ss) in src_runs:
                        zs = gath(qs, "zs", Z, ss, cb, nch, qi, "qsi", t_qsrc)
                        for (cb2, nch2, ds) in [x for x in dst_runs
                                                if cb <= x[0] < cb + nch]:
                            zd = gath(qd, "zd", Z, ds, cb2, nch2, qi, "qdi", t_qdst)
                            prod = qd.tile([P, nch2, OUT], FP, tag="prod")
                            nc.vector.tensor_tensor(
                                out=prod[:], in0=zs[:, cb2 - cb:cb2 - cb + nch2, :],
                                in1=zd[:], op=mybir.AluOpType.mult)
                            nc.vector.tensor_reduce(
                                out=red_all[:, cb2:cb2 + nch2], in_=prod[:],
                                axis=mybir.AxisListType.X, op=mybir.AluOpType.add)
                    nc.sync.dma_start(out=o_dec[:], in_=red_all[:])

    nc.compile()
    return nc


_BUILD_CACHE = {}


def _meta_key(meta):
    import hashlib
    h = hashlib.sha256()
    h.update(np.asarray(meta["chunk_win"]).tobytes())
    h.update(np.asarray(meta["chunk_seg"]).tobytes())
    h.update(np.asarray(meta["st_seg_nchunks"]).tobytes())
    h.update(repr((meta["NCH"], meta["NQCH"], meta["src_runs"], meta["dst_runs"])).encode())
    return h.hexdigest()


def kernel(**inputs):
    in_maps, meta = _preprocess(inputs)
    key = _meta_key(meta)
    if key not in _BUILD_CACHE:
        _BUILD_CACHE[key] = _build(meta)
    nc = _BUILD_CACHE[key]
    res = run_bass_kernel_spmd(nc, in_maps, core_ids=list(range(NCORES)))
    kernel.last_results = res

    out = np.zeros(EQ, dtype=np.float32)
    perm_dec = meta["perm_dec"]
    NQCH = meta["NQCH"]
    for r in range(NCORES):
        od = res.results[r]["out"]          # [128, NQCH]
        pr = perm_dec[r]                    # [NQCH, 128] slot -> local query
        valid = pr >= 0
        out[r * EQC + pr[valid]] = od.T[valid]
    return out

